# revision 3
# baseline (speedup 1.0000x reference)
"""ExpressionAttentionLayer Trainium2 kernel.

Math (per reference, algebraically folded):
  fused/q/k projections folded on the host into one [1024,128] weight per
  core; A_bar = softmax(qk)*M / L1 == exp(qk)*M / sum_k(exp(qk)*M) (the
  softmax denominator cancels; logits are tiny so no max-subtraction);
  the key-sum denominator rides as a ones-column appended to V.

Device decomposition: core d = batch d//4, head pair (2*(d%4), 2*(d%4)+1).

Structure (~1.2x over the previous kernel; ACT exp stream is the
bottleneck at ~1.11us/tile x 64 tiles):
  * One flat software pipeline over all 64 (qb, kt) tiles, no per-qb
    barrier: ST pair (row-tiled, concurrent) -> exp [128,1024] -> fused
    broadcast-AP mask multiply -> AV pair lagging LAG tiles. All matmul
    operands bf16.
  * xt staged token-group-major on the sync/HWDGE queue so group 0's
    q^T/k^T close early; later groups' chains emitted inside the loop.
  * Mask pre-tiled on the host to one contiguous 128KB block per tile.
  * Per-qb epilogue off the critical path: numerators copied out of PSUM
    immediately (frees the 8-bank budget: st 2x2 + qk 2 + num 2), 1/den
    broadcast via a small DRAM bounce on spread-out schedule slots,
    projections interleaved into the next qb's tile stream.
  * Last-qb tail: unscaled per-head projections start immediately; 1/den
    is transposed to per-partition layout with 8 rank-1 PE matmuls (no
    DRAM roundtrip) and applied as a per-partition scale split ACT/DVE.
"""

import os
import sys
from collections import defaultdict

for _p in ("/opt/trn_rl_repo", "/root/.axon_site/_ro/trn_rl_repo"):
    if os.path.isdir(_p) and _p not in sys.path:
        sys.path.insert(0, _p)

import numpy as np

import concourse.bass as bass
import concourse.mybir as mybir
import concourse.tile as tile
from concourse import bacc
from concourse.bass_utils import run_bass_kernel_spmd

B, S, D, H, HD = 2, 2048, 512, 8, 64
KX = 2 * D
NCH = KX // 128
NG = 4
N_CORES = 8
QB = 512
NQB = S // QB
KT = 128
NKT = S // KT
NT = NQB * NKT
LAG = 4
PF = 7
SCALE = 1.0 / np.sqrt(HD)

f32 = mybir.dt.float32
f32r = mybir.dt.float32r
bf16 = mybir.dt.bfloat16

M_DT = f32r if os.environ.get("KERNEL_M_DT", "bf16") == "f32r" else bf16
P_DT = f32r if os.environ.get("KERNEL_P_DT", "bf16") == "f32r" else bf16
X_DT = f32r if os.environ.get("KERNEL_X_DT", "bf16") == "f32r" else bf16
FUSED_MASK = os.environ.get("KERNEL_FUSED_MASK", "1") == "1"
OUT_PSUM = os.environ.get("KERNEL_OUT_PSUM", "0") == "1"

_compiled = None
_last_results = None


def _build():
    nc = bacc.Bacc("TRN2", target_bir_lowering=False, debug=False,
                   num_devices=N_CORES)
    AF = mybir.ActivationFunctionType

    xt = nc.dram_tensor("xt", [128, NG * NCH, QB], X_DT,
                        kind="ExternalInput").ap()
    # mask pre-tiled on the host: tile T = (qb*16+kt) is one contiguous
    # 128x512 block, so mask DMA moves 128 KB of contiguous DRAM per tile
    # instead of 128 strided 1 KB lines (which capped DMA at ~130 GB/s).
    mt = nc.dram_tensor("mt", [NT, KT, QB], M_DT, kind="ExternalInput").ap()
    wq = nc.dram_tensor("wq", [128, NCH, 128], X_DT, kind="ExternalInput").ap()
    wk = nc.dram_tensor("wk", [128, NCH, 128], X_DT, kind="ExternalInput").ap()
    bq = nc.dram_tensor("bq", [128, 1], f32, kind="ExternalInput").ap()
    bk = nc.dram_tensor("bk", [128, 1], f32, kind="ExternalInput").ap()
    v0 = nc.dram_tensor("v0", [128, NKT, HD + 1], P_DT, kind="ExternalInput").ap()
    v1 = nc.dram_tensor("v1", [128, NKT, HD + 1], P_DT, kind="ExternalInput").ap()
    wo0 = nc.dram_tensor("wo0", [HD, D], bf16, kind="ExternalInput").ap()
    wo1 = nc.dram_tensor("wo1", [HD, D], bf16, kind="ExternalInput").ap()
    out = nc.dram_tensor("out", [S, D], f32, kind="ExternalOutput").ap()

    with tile.TileContext(nc) as tc:
        with tc.tile_pool(name="const", bufs=1) as const, \
             tc.tile_pool(name="mtp", bufs=10) as mtp, \
             tc.tile_pool(name="ep", bufs=6) as ep, \
             tc.tile_pool(name="pp", bufs=8) as pp, \
             tc.tile_pool(name="nsb", bufs=2) as nsb, \
             tc.tile_pool(name="small", bufs=2) as small, \
             tc.tile_pool(name="bcp", bufs=2) as bcp, \
             tc.tile_pool(name="shp", bufs=2) as shp, \
             tc.tile_pool(name="outp", bufs=2) as outp, \
             tc.tile_pool(name="pst", bufs=2, space="PSUM") as pst, \
             tc.tile_pool(name="pacc", bufs=2, space="PSUM") as pacc, \
             tc.tile_pool(name="drp", bufs=2, space="DRAM") as drp:

            # ---- PE warm-up burst + Exp table preload -----------------
            warm_in = const.tile([128, QB], bf16)
            nc.vector.memset(warm_in, 1.0)
            one_f = const.tile([1, 1], f32)
            nc.vector.memset(one_f, 1.0)
            warm_o = const.tile([1, 8], f32)
            for i in range(13):
                warm_ps = pacc.tile([128, QB], f32, tag="qk",
                                    name=f"warm{i}")
                nc.tensor.matmul(warm_ps, warm_in[:, 0:128], warm_in,
                                 start=True, stop=True)
            nc.scalar.activation(warm_o, warm_in[0:1, 0:8], AF.Exp)

            # ---- input DMAs -------------------------------------------
            # All on the sync (HWDGE) queue: the gpsimd SWDGE queue was
            # observed to start transfers ~7us late. xt groups lead, since
            # the whole pipeline starts when group 0 lands.
            xt_s = const.tile([128, NG * NCH, QB], X_DT)

            def issue_xt(g):
                nc.sync.dma_start(out=xt_s[:, g * NCH:(g + 1) * NCH, :],
                                  in_=xt[:, g * NCH:(g + 1) * NCH, :])

            mt_tiles = {}

            def issue_mask(T):
                m = mtp.tile([128, QB], M_DT, tag="mt", name="mt_t")
                nc.sync.dma_start(out=m, in_=mt[T, :, :])
                mt_tiles[T] = m

            issue_xt(0)
            wq_s = const.tile([128, NCH, 128], X_DT)
            wk_s = const.tile([128, NCH, 128], X_DT)
            nc.sync.dma_start(out=wq_s, in_=wq)
            nc.sync.dma_start(out=wk_s, in_=wk)
            bq_s = const.tile([128, 1], f32)
            bk_s = const.tile([128, 1], f32)
            nc.sync.dma_start(out=bq_s, in_=bq)
            nc.sync.dma_start(out=bk_s, in_=bk)
            issue_xt(1)
            issue_mask(0)
            issue_mask(1)
            v0_s = const.tile([128, NKT, HD + 1], P_DT)
            v1_s = const.tile([128, NKT, HD + 1], P_DT)
            nc.sync.dma_start(out=v0_s, in_=v0)
            nc.sync.dma_start(out=v1_s, in_=v1)
            issue_xt(2)
            issue_mask(2)
            issue_mask(3)
            issue_xt(3)
            for T in range(4, PF):
                issue_mask(T)
            wo0_s = const.tile([HD, D], bf16)
            wo1_s = const.tile([HD, D], bf16)
            nc.sync.dma_start(out=wo0_s, in_=wo0)
            nc.sync.dma_start(out=wo1_s, in_=wo1)

            qT_g = [const.tile([128, QB], bf16, name=f"qT{g}")
                    for g in range(NG)]
            kT_g = [const.tile([128, QB], bf16, name=f"kT{g}")
                    for g in range(NG)]

            def emit_group(g):
                for t, w_s, b_s, dst in ((0, wq_s, bq_s, qT_g[g]),
                                         (1, wk_s, bk_s, kT_g[g])):
                    acc = pacc.tile([128, QB], f32, tag="qk",
                                    name=f"qk{t}_{g}")
                    for ch in range(NCH):
                        nc.tensor.matmul(acc, w_s[:, ch, :],
                                         xt_s[:, g * NCH + ch, :],
                                         start=(ch == 0),
                                         stop=(ch == NCH - 1))
                    nc.vector.tensor_scalar_add(dst, acc, b_s)

            # ---- pipeline body helpers --------------------------------
            p_tiles = {}
            num = {}

            def emit_st(T):
                qb, kt = divmod(T, NKT)
                g, c = divmod(kt, NG)
                st = pst.tile([128, 2 * QB], f32, tag="st", name="st")
                for h in range(2):
                    nc.tensor.matmul(
                        st[:, h * QB:(h + 1) * QB],
                        kT_g[g][h * HD:(h + 1) * HD, c * KT:(c + 1) * KT],
                        qT_g[qb][h * HD:(h + 1) * HD, :],
                        start=True, stop=True,
                        tile_position=(h * HD, 0))
                e_t = ep.tile([128, 2 * QB], P_DT, tag="e", name="e_t")
                nc.scalar.activation(e_t, st, AF.Exp)
                p_t = pp.tile([128, 2 * QB], P_DT, tag="p", name="p_t")
                m = mt_tiles.pop(T)
                if FUSED_MASK:
                    mb = bass.AP(tensor=m.tensor, offset=m.offset,
                                 ap=[list(m.ap[0]), [0, 2], list(m.ap[1])])
                    e3 = bass.AP(tensor=e_t.tensor, offset=e_t.offset,
                                 ap=[list(e_t.ap[0]), [QB, 2], [1, QB]])
                    p3 = bass.AP(tensor=p_t.tensor, offset=p_t.offset,
                                 ap=[list(p_t.ap[0]), [QB, 2], [1, QB]])
                    nc.vector.tensor_mul(p3, e3, mb)
                else:
                    for h in range(2):
                        nc.vector.tensor_mul(
                            p_t[:, h * QB:(h + 1) * QB],
                            e_t[:, h * QB:(h + 1) * QB], m)
                p_tiles[T] = p_t

            def emit_av(T):
                qb, kt = divmod(T, NKT)
                if kt == 0:
                    num[qb] = [pacc.tile([HD + 1, QB], f32, tag="num",
                                         name=f"num{h}_{qb}")
                               for h in range(2)]
                p_t = p_tiles.pop(T)
                for h, v_s in ((0, v0_s), (1, v1_s)):
                    nc.tensor.matmul(num[qb][h], v_s[:, kt, :],
                                     p_t[:, h * QB:(h + 1) * QB],
                                     start=(kt == 0), stop=(kt == NKT - 1))

            def stage1(qb):
                """At qb's last AV: drain numerators out of PSUM, start the
                1/den bounce. Returns state for the deferred stages."""
                n0, n1 = num.pop(qb)
                st8 = {}
                for h, n in ((0, n0), (1, n1)):
                    den = small.tile([1, QB], f32, tag=f"den{h}", name="den")
                    nc.vector.tensor_copy(den, n[HD:HD + 1, :])
                    ns = nsb.tile([HD, QB], f32, tag=f"nsb{h}",
                                  name=f"nsb{h}")
                    nc.vector.tensor_copy(ns, n[0:HD, :])
                    rec = small.tile([1, QB], f32, tag=f"rec{h}", name="rec")
                    nc.vector.reciprocal_approx_fast(rec, den)
                    rec_d = drp.tile([1, QB], f32, tag=f"recd{h}",
                                     name="rec_d")
                    nc.sync.dma_start(out=rec_d, in_=rec)
                    bc = bcp.tile([HD, QB], f32, tag=f"bc{h}", name="bc")
                    rb = bass.AP(tensor=rec_d.tensor, offset=rec_d.offset,
                                 ap=[[0, HD], [1, QB]])
                    nc.sync.dma_start(out=bc, in_=rb)
                    st8[h] = (ns, bc)
                return st8

            def make_sh(st8, h, shs):
                def cl():
                    ns, bc = st8[h]
                    sh = shp.tile([HD, QB], bf16, tag=f"sh{h}", name=f"sh{h}")
                    nc.vector.tensor_mul(sh, ns, bc)
                    shs[h] = sh
                return cl

            def make_proj(qb, shs, blk):
                def cl():
                    pr = pacc.tile([128, D], f32, tag="qk", name="pr")
                    nc.tensor.matmul(pr, shs[0][:, blk * 128:(blk + 1) * 128],
                                     wo0_s, start=True, stop=False)
                    nc.tensor.matmul(pr, shs[1][:, blk * 128:(blk + 1) * 128],
                                     wo1_s, start=False, stop=True)
                    rows = slice(qb * QB + blk * 128,
                                 qb * QB + (blk + 1) * 128)
                    if OUT_PSUM:
                        nc.sync.dma_start(out=out[rows, :], in_=pr)
                    else:
                        o_t = outp.tile([128, D], f32, tag="o", name="o_t")
                        nc.vector.tensor_copy(o_t, pr)
                        nc.sync.dma_start(out=out[rows, :], in_=o_t)
                return cl

            # ---- the flat pipeline ------------------------------------
            schedule = defaultdict(list)
            for T in range(NT + LAG):
                if T < NT:
                    qb, kt = divmod(T, NKT)
                    if qb == 0 and kt % 4 == 0:
                        emit_group(kt // 4)
                    if T + PF < NT:
                        issue_mask(T + PF)
                    emit_st(T)
                for cl in schedule.pop(T, []):
                    cl()
                if T >= LAG:
                    TT = T - LAG
                    emit_av(TT)
                    qb2, kt2 = divmod(TT, NKT)
                    if kt2 == NKT - 1 and qb2 < NQB - 1:
                        st8 = stage1(qb2)
                        shs = {}
                        schedule[T + 2].append(make_sh(st8, 0, shs))
                        schedule[T + 3].append(make_sh(st8, 1, shs))
                        for b in range(4):
                            schedule[T + 5 + 2 * b].append(
                                make_proj(qb2, shs, b))

            # ---- tail: last qb, latency-optimized ---------------------
            # Unscaled per-head projections start as soon as the numerators
            # are copied out (keeps the PE warm); 1/den is transposed into
            # per-partition layout with 8 rank-1 PE matmuls (no DRAM bounce)
            # and applied as a per-partition scale, split across ACT + DVE.
            qb = NQB - 1
            n0, n1 = num.pop(qb)
            dd = small.tile([1, 2 * QB], f32, tag="dd", name="dd")
            nc.vector.tensor_copy(dd[:, 0:QB], n0[HD:HD + 1, :])
            nc.vector.tensor_copy(dd[:, QB:2 * QB], n1[HD:HD + 1, :])
            rec = small.tile([1, 2 * QB], f32, tag="rec2", name="rec")
            nc.vector.reciprocal_approx_fast(rec, dd)
            nss = []
            for h, n in ((0, n0), (1, n1)):
                ns = nsb.tile([HD, QB], bf16, tag=f"nst{h}", name=f"nst{h}")
                nc.vector.tensor_copy(ns, n[0:HD, :])
                nss.append(ns)
            prs = []
            for b in (0, 1):
                st_blk = pst.tile([128, 2 * QB], f32, tag="st", name="st_pr")
                prs.append((st_blk[:, 0:QB], st_blk[:, QB:2 * QB]))
            q0 = pacc.tile([128, D], f32, tag="qk", name="prq0")
            q1 = pacc.tile([128, D], f32, tag="qk", name="prq1")
            prs.append((q0, q1))

            def proj_mm(b):
                for h, ns, wo_s in ((0, nss[0], wo0_s), (1, nss[1], wo1_s)):
                    nc.tensor.matmul(prs[b][h],
                                     ns[:, b * 128:(b + 1) * 128],
                                     wo_s, start=True, stop=True)

            def scale_blk(b):
                t0 = small.tile([128, D], f32, tag="t0", name="t0")
                nc.scalar.mul(t0, prs[b][0], rec_t[:, b:b + 1])
                o_t = outp.tile([128, D], f32, tag="o", name="o_t")
                nc.vector.scalar_tensor_tensor(
                    o_t, prs[b][1], rec_t[:, 4 + b:4 + b + 1], t0,
                    mybir.AluOpType.mult, mybir.AluOpType.add)
                rows = slice(qb * QB + b * 128, qb * QB + (b + 1) * 128)
                nc.sync.dma_start(out=out[rows, :], in_=o_t)

            proj_mm(0)
            proj_mm(1)
            proj_mm(2)
            rec_tp = pacc.tile([128, 8], f32, tag="num", name="rec_tp")
            for c in range(8):
                nc.tensor.matmul(rec_tp[:, c:c + 1],
                                 rec[:, c * 128:(c + 1) * 128], one_f,
                                 start=True, stop=True)
            rec_t = small.tile([128, 8], f32, tag="rect", name="rec_t")
            nc.vector.tensor_copy(rec_t, rec_tp)
            scale_blk(0)
            st_blk = pst.tile([128, 2 * QB], f32, tag="st", name="st_pr")
            prs.append((st_blk[:, 0:QB], st_blk[:, QB:2 * QB]))
            proj_mm(3)
            scale_blk(1)
            scale_blk(2)
            scale_blk(3)

    nc.compile()
    return nc


def _get_compiled():
    global _compiled
    if _compiled is None:
        _compiled = _build()
    return _compiled


def kernel(gene_emb, expr_emb, V, M, fused_W, fused_b, Wq, bq, Wk, bk,
           out_W, out_b):
    gene_emb = np.asarray(gene_emb, dtype=np.float32)
    expr_emb = np.asarray(expr_emb, dtype=np.float32)
    V = np.asarray(V, dtype=np.float32)
    M = np.asarray(M, dtype=np.float32)
    fused_W = np.asarray(fused_W, dtype=np.float32)
    fused_b = np.asarray(fused_b, dtype=np.float32)
    Wq_ = np.asarray(Wq, dtype=np.float32)
    bq_ = np.asarray(bq, dtype=np.float32)
    Wk_ = np.asarray(Wk, dtype=np.float32)
    bk_ = np.asarray(bk, dtype=np.float32)
    out_W = np.asarray(out_W, dtype=np.float32)
    out_b = np.asarray(out_b, dtype=np.float32)

    nc = _get_compiled()

    import ml_dtypes
    m_np = np.float32 if M_DT == f32r else ml_dtypes.bfloat16
    p_np = np.float32 if P_DT == f32r else ml_dtypes.bfloat16
    x_np = np.float32 if X_DT == f32r else ml_dtypes.bfloat16

    fW = fused_W.astype(np.float64)
    Wqc = (fW @ Wq_.astype(np.float64)) * SCALE
    bqc = (fused_b.astype(np.float64) @ Wq_.astype(np.float64) + bq_) * SCALE
    Wkc = fW @ Wk_.astype(np.float64)
    bkc = fused_b.astype(np.float64) @ Wk_.astype(np.float64) + bk_

    def chunk_major(a, nch):  # [nch*128, F] -> [128, nch, F]
        F = a.shape[1]
        return np.ascontiguousarray(
            a.reshape(nch, 128, F).transpose(1, 0, 2))

    xt_b, mt_b = [], []
    for b in range(B):
        XT = np.concatenate([gene_emb[b], expr_emb[b]], axis=-1).T  # [1024,S]
        xg = XT.reshape(NCH, 128, NG, QB).transpose(1, 2, 0, 3)
        xt_b.append(np.ascontiguousarray(
            xg.reshape(128, NG * NCH, QB)).astype(x_np))
        # pre-tile the transposed mask: [T=(qb,kt), 128 keys, 512 queries]
        mtt = M[b].T.reshape(NKT, KT, NQB, QB).transpose(2, 0, 1, 3)
        mt_b.append(np.ascontiguousarray(
            mtt.reshape(NT, KT, QB)).astype(m_np))

    ones_col = np.ones((S, 1), np.float32)
    in_maps = []
    for d in range(N_CORES):
        b, p = d // 4, d % 4
        h0 = 2 * p
        cols = slice(p * 128, (p + 1) * 128)
        vs = []
        for h in (h0, h0 + 1):
            Vh = np.concatenate([V[b, :, h, :], ones_col], axis=1)  # [S,65]
            vs.append(chunk_major(Vh, NKT).astype(p_np))
        in_maps.append({
            "xt": xt_b[b],
            "mt": mt_b[b],
            "wq": chunk_major(Wqc[:, cols].astype(np.float32),
                              NCH).astype(x_np),
            "wk": chunk_major(Wkc[:, cols].astype(np.float32),
                              NCH).astype(x_np),
            "bq": bqc[cols].astype(np.float32).reshape(128, 1),
            "bk": bkc[cols].astype(np.float32).reshape(128, 1),
            "v0": vs[0],
            "v1": vs[1],
            "wo0": np.ascontiguousarray(
                out_W[h0 * HD:(h0 + 1) * HD, :]).astype(ml_dtypes.bfloat16),
            "wo1": np.ascontiguousarray(
                out_W[(h0 + 1) * HD:(h0 + 2) * HD, :]).astype(
                    ml_dtypes.bfloat16),
        })

    global _last_results
    n_run = int(os.environ.get("KERNEL_CORES", N_CORES))
    if n_run < N_CORES:
        in_maps = in_maps[:1] * N_CORES
    res = run_bass_kernel_spmd(nc, in_maps[:n_run],
                               core_ids=list(range(n_run)))
    if n_run < N_CORES:
        res.results = list(res.results) * (N_CORES // n_run)
    _last_results = res

    final = np.broadcast_to(out_b, (B, S, D)).astype(np.float32).copy()
    for d in range(N_CORES):
        final[d // 4] += res.results[d]["out"]
    return final


# revision 4
# speedup vs baseline: 1.0400x; 1.0400x over previous
"""ExpressionAttentionLayer Trainium2 kernel.

Math (per reference, algebraically folded):
  fused/q/k projections folded on the host into one [1024,128] weight per
  core; A_bar = softmax(qk)*M / L1 == exp(qk)*M / sum_k(exp(qk)*M) (the
  softmax denominator cancels; logits are tiny so no max-subtraction);
  the key-sum denominator rides as a ones-column appended to V.

Device decomposition: core d = batch d//4, head pair (2*(d%4), 2*(d%4)+1).

Structure (~1.2x over the previous kernel; ACT exp stream is the
bottleneck at ~1.11us/tile x 64 tiles):
  * One flat software pipeline over all 64 (qb, kt) tiles, no per-qb
    barrier: ST pair (row-tiled, concurrent) -> exp [128,1024] -> fused
    broadcast-AP mask multiply -> AV pair lagging LAG tiles. All matmul
    operands bf16.
  * xt staged token-group-major on the sync/HWDGE queue so group 0's
    q^T/k^T close early; later groups' chains emitted inside the loop.
  * Mask pre-tiled on the host to one contiguous 128KB block per tile.
  * Per-qb epilogue off the critical path: numerators copied out of PSUM
    immediately (frees the 8-bank budget: st 2x2 + qk 2 + num 2), 1/den
    broadcast via a small DRAM bounce on spread-out schedule slots,
    projections interleaved into the next qb's tile stream.
  * Last-qb tail: unscaled per-head projections start immediately; 1/den
    is transposed to per-partition layout with 8 rank-1 PE matmuls (no
    DRAM roundtrip) and applied as a per-partition scale split ACT/DVE.
"""

import os
import sys
from collections import defaultdict

for _p in ("/opt/trn_rl_repo", "/root/.axon_site/_ro/trn_rl_repo"):
    if os.path.isdir(_p) and _p not in sys.path:
        sys.path.insert(0, _p)

import numpy as np

import concourse.bass as bass
import concourse.mybir as mybir
import concourse.tile as tile
from concourse import bacc
from concourse.bass_utils import run_bass_kernel_spmd

B, S, D, H, HD = 2, 2048, 512, 8, 64
KX = 2 * D
NCH = KX // 128
NG = 4
N_CORES = 8
QB = 512
NQB = S // QB
KT = 128
NKT = S // KT
NT = NQB * NKT
LAG = 4
PF = 6
SCALE = 1.0 / np.sqrt(HD)

f32 = mybir.dt.float32
f32r = mybir.dt.float32r
bf16 = mybir.dt.bfloat16

M_DT = f32r if os.environ.get("KERNEL_M_DT", "bf16") == "f32r" else bf16
P_DT = f32r if os.environ.get("KERNEL_P_DT", "bf16") == "f32r" else bf16
X_DT = f32r if os.environ.get("KERNEL_X_DT", "bf16") == "f32r" else bf16
FUSED_MASK = os.environ.get("KERNEL_FUSED_MASK", "1") == "1"
OUT_PSUM = os.environ.get("KERNEL_OUT_PSUM", "0") == "1"

_compiled = None
_last_results = None


def _build():
    nc = bacc.Bacc("TRN2", target_bir_lowering=False, debug=False,
                   num_devices=N_CORES)
    AF = mybir.ActivationFunctionType

    xt = nc.dram_tensor("xt", [128, NG * NCH, QB], X_DT,
                        kind="ExternalInput").ap()
    # mask pre-tiled on the host: tile T = (qb*16+kt) is one contiguous
    # 128x512 block, so mask DMA moves 128 KB of contiguous DRAM per tile
    # instead of 128 strided 1 KB lines (which capped DMA at ~130 GB/s).
    mt = nc.dram_tensor("mt", [NT, KT, QB], M_DT, kind="ExternalInput").ap()
    wq = nc.dram_tensor("wq", [128, NCH, 128], X_DT, kind="ExternalInput").ap()
    wk = nc.dram_tensor("wk", [128, NCH, 128], X_DT, kind="ExternalInput").ap()
    bq = nc.dram_tensor("bq", [128, 1], f32, kind="ExternalInput").ap()
    bk = nc.dram_tensor("bk", [128, 1], f32, kind="ExternalInput").ap()
    v0 = nc.dram_tensor("v0", [128, NKT, HD + 1], P_DT, kind="ExternalInput").ap()
    v1 = nc.dram_tensor("v1", [128, NKT, HD + 1], P_DT, kind="ExternalInput").ap()
    wo0 = nc.dram_tensor("wo0", [HD, D], bf16, kind="ExternalInput").ap()
    wo1 = nc.dram_tensor("wo1", [HD, D], bf16, kind="ExternalInput").ap()
    out = nc.dram_tensor("out", [S, D], f32, kind="ExternalOutput").ap()

    with tile.TileContext(nc) as tc:
        with tc.tile_pool(name="const", bufs=1) as const, \
             tc.tile_pool(name="mtp", bufs=10) as mtp, \
             tc.tile_pool(name="ep", bufs=6) as ep, \
             tc.tile_pool(name="pp", bufs=8) as pp, \
             tc.tile_pool(name="nsb", bufs=2) as nsb, \
             tc.tile_pool(name="small", bufs=2) as small, \
             tc.tile_pool(name="bcp", bufs=2) as bcp, \
             tc.tile_pool(name="shp", bufs=2) as shp, \
             tc.tile_pool(name="outp", bufs=2) as outp, \
             tc.tile_pool(name="pst", bufs=2, space="PSUM") as pst, \
             tc.tile_pool(name="pacc", bufs=2, space="PSUM") as pacc, \
             tc.tile_pool(name="drp", bufs=2, space="DRAM") as drp:

            # ---- PE warm-up burst + Exp table preload -----------------
            warm_in = const.tile([128, QB], bf16)
            nc.vector.memset(warm_in, 1.0)
            one_f = const.tile([1, 1], f32)
            nc.vector.memset(one_f, 1.0)
            warm_o = const.tile([1, 8], f32)
            for i in range(13):
                warm_ps = pacc.tile([128, QB], f32, tag="qk",
                                    name=f"warm{i}")
                nc.tensor.matmul(warm_ps, warm_in[:, 0:128], warm_in,
                                 start=True, stop=True)
            nc.scalar.activation(warm_o, warm_in[0:1, 0:8], AF.Exp)

            # ---- input DMAs -------------------------------------------
            # All on the sync (HWDGE) queue: the gpsimd SWDGE queue was
            # observed to start transfers ~7us late. xt groups lead, since
            # the whole pipeline starts when group 0 lands.
            xt_s = const.tile([128, NG * NCH, QB], X_DT)

            def issue_xt(g):
                nc.sync.dma_start(out=xt_s[:, g * NCH:(g + 1) * NCH, :],
                                  in_=xt[:, g * NCH:(g + 1) * NCH, :])

            mt_tiles = {}

            def issue_mask(T):
                m = mtp.tile([128, QB], M_DT, tag="mt", name="mt_t")
                nc.sync.dma_start(out=m, in_=mt[T, :, :])
                mt_tiles[T] = m

            issue_xt(0)
            wq_s = const.tile([128, NCH, 128], X_DT)
            wk_s = const.tile([128, NCH, 128], X_DT)
            nc.sync.dma_start(out=wq_s, in_=wq)
            nc.sync.dma_start(out=wk_s, in_=wk)
            bq_s = const.tile([128, 1], f32)
            bk_s = const.tile([128, 1], f32)
            nc.sync.dma_start(out=bq_s, in_=bq)
            nc.sync.dma_start(out=bk_s, in_=bk)
            issue_xt(1)
            issue_mask(0)
            issue_mask(1)
            v0_s = const.tile([128, NKT, HD + 1], P_DT)
            v1_s = const.tile([128, NKT, HD + 1], P_DT)
            nc.sync.dma_start(out=v0_s, in_=v0)
            nc.sync.dma_start(out=v1_s, in_=v1)
            issue_xt(2)
            issue_mask(2)
            issue_mask(3)
            issue_xt(3)
            for T in range(4, PF):
                issue_mask(T)
            wo0_s = const.tile([HD, D], bf16)
            wo1_s = const.tile([HD, D], bf16)
            nc.sync.dma_start(out=wo0_s, in_=wo0)
            nc.sync.dma_start(out=wo1_s, in_=wo1)

            qT_g = [const.tile([128, QB], bf16, name=f"qT{g}")
                    for g in range(NG)]
            kT_g = [const.tile([128, QB], bf16, name=f"kT{g}")
                    for g in range(NG)]

            def emit_group(g):
                for t, w_s, b_s, dst in ((0, wq_s, bq_s, qT_g[g]),
                                         (1, wk_s, bk_s, kT_g[g])):
                    acc = pacc.tile([128, QB], f32, tag="qk",
                                    name=f"qk{t}_{g}")
                    for ch in range(NCH):
                        nc.tensor.matmul(acc, w_s[:, ch, :],
                                         xt_s[:, g * NCH + ch, :],
                                         start=(ch == 0),
                                         stop=(ch == NCH - 1))
                    nc.vector.tensor_scalar_add(dst, acc, b_s)

            # ---- pipeline body helpers --------------------------------
            p_tiles = {}
            num = {}

            def emit_st(T):
                qb, kt = divmod(T, NKT)
                g, c = divmod(kt, NG)
                st = pst.tile([128, 2 * QB], f32, tag="st", name="st")
                for h in range(2):
                    nc.tensor.matmul(
                        st[:, h * QB:(h + 1) * QB],
                        kT_g[g][h * HD:(h + 1) * HD, c * KT:(c + 1) * KT],
                        qT_g[qb][h * HD:(h + 1) * HD, :],
                        start=True, stop=True,
                        tile_position=(h * HD, 0))
                e_t = ep.tile([128, 2 * QB], P_DT, tag="e", name="e_t")
                nc.scalar.activation(e_t, st, AF.Exp)
                p_t = pp.tile([128, 2 * QB], P_DT, tag="p", name="p_t")
                m = mt_tiles.pop(T)
                if FUSED_MASK:
                    mb = bass.AP(tensor=m.tensor, offset=m.offset,
                                 ap=[list(m.ap[0]), [0, 2], list(m.ap[1])])
                    e3 = bass.AP(tensor=e_t.tensor, offset=e_t.offset,
                                 ap=[list(e_t.ap[0]), [QB, 2], [1, QB]])
                    p3 = bass.AP(tensor=p_t.tensor, offset=p_t.offset,
                                 ap=[list(p_t.ap[0]), [QB, 2], [1, QB]])
                    nc.vector.tensor_mul(p3, e3, mb)
                else:
                    for h in range(2):
                        nc.vector.tensor_mul(
                            p_t[:, h * QB:(h + 1) * QB],
                            e_t[:, h * QB:(h + 1) * QB], m)
                p_tiles[T] = p_t

            def emit_av(T):
                qb, kt = divmod(T, NKT)
                if kt == 0:
                    num[qb] = [pacc.tile([HD + 1, QB], f32, tag="num",
                                         name=f"num{h}_{qb}")
                               for h in range(2)]
                p_t = p_tiles.pop(T)
                for h, v_s in ((0, v0_s), (1, v1_s)):
                    nc.tensor.matmul(num[qb][h], v_s[:, kt, :],
                                     p_t[:, h * QB:(h + 1) * QB],
                                     start=(kt == 0), stop=(kt == NKT - 1))

            def stage1(qb):
                """At qb's last AV: drain numerators out of PSUM, start the
                1/den bounce. Returns state for the deferred stages."""
                n0, n1 = num.pop(qb)
                st8 = {}
                for h, n in ((0, n0), (1, n1)):
                    den = small.tile([1, QB], f32, tag=f"den{h}", name="den")
                    nc.vector.tensor_copy(den, n[HD:HD + 1, :])
                    ns = nsb.tile([HD, QB], f32, tag=f"nsb{h}",
                                  name=f"nsb{h}")
                    nc.vector.tensor_copy(ns, n[0:HD, :])
                    rec = small.tile([1, QB], f32, tag=f"rec{h}", name="rec")
                    nc.vector.reciprocal_approx_fast(rec, den)
                    rec_d = drp.tile([1, QB], f32, tag=f"recd{h}",
                                     name="rec_d")
                    nc.sync.dma_start(out=rec_d, in_=rec)
                    bc = bcp.tile([HD, QB], f32, tag=f"bc{h}", name="bc")
                    rb = bass.AP(tensor=rec_d.tensor, offset=rec_d.offset,
                                 ap=[[0, HD], [1, QB]])
                    nc.sync.dma_start(out=bc, in_=rb)
                    st8[h] = (ns, bc)
                return st8

            def make_sh(st8, h, shs):
                def cl():
                    ns, bc = st8[h]
                    sh = shp.tile([HD, QB], bf16, tag=f"sh{h}", name=f"sh{h}")
                    nc.vector.tensor_mul(sh, ns, bc)
                    shs[h] = sh
                return cl

            def make_proj(qb, shs, blk):
                def cl():
                    pr = pacc.tile([128, D], f32, tag="qk", name="pr")
                    nc.tensor.matmul(pr, shs[0][:, blk * 128:(blk + 1) * 128],
                                     wo0_s, start=True, stop=False)
                    nc.tensor.matmul(pr, shs[1][:, blk * 128:(blk + 1) * 128],
                                     wo1_s, start=False, stop=True)
                    rows = slice(qb * QB + blk * 128,
                                 qb * QB + (blk + 1) * 128)
                    if OUT_PSUM:
                        nc.sync.dma_start(out=out[rows, :], in_=pr)
                    else:
                        o_t = outp.tile([128, D], f32, tag="o", name="o_t")
                        nc.vector.tensor_copy(o_t, pr)
                        nc.sync.dma_start(out=out[rows, :], in_=o_t)
                return cl

            # ---- the flat pipeline ------------------------------------
            schedule = defaultdict(list)
            for T in range(NT + LAG):
                if T < NT:
                    qb, kt = divmod(T, NKT)
                    if qb == 0 and kt % 4 == 0:
                        emit_group(kt // 4)
                    if T + PF < NT:
                        issue_mask(T + PF)
                    emit_st(T)
                for cl in schedule.pop(T, []):
                    cl()
                if T >= LAG:
                    TT = T - LAG
                    emit_av(TT)
                    qb2, kt2 = divmod(TT, NKT)
                    if kt2 == NKT - 1 and qb2 < NQB - 1:
                        st8 = stage1(qb2)
                        shs = {}
                        schedule[T + 2].append(make_sh(st8, 0, shs))
                        schedule[T + 3].append(make_sh(st8, 1, shs))
                        for b in range(4):
                            schedule[T + 5 + 2 * b].append(
                                make_proj(qb2, shs, b))

            # ---- tail: last qb, latency-optimized ---------------------
            # Unscaled per-head projections start as soon as the numerators
            # are copied out (keeps the PE warm); 1/den is transposed into
            # per-partition layout with 8 rank-1 PE matmuls (no DRAM bounce)
            # and applied as a per-partition scale, split across ACT + DVE.
            qb = NQB - 1
            n0, n1 = num.pop(qb)
            dd = small.tile([1, 2 * QB], f32, tag="dd", name="dd")
            nc.vector.tensor_copy(dd[:, 0:QB], n0[HD:HD + 1, :])
            nc.vector.tensor_copy(dd[:, QB:2 * QB], n1[HD:HD + 1, :])
            rec = small.tile([1, 2 * QB], f32, tag="rec2", name="rec")
            nc.vector.reciprocal_approx_fast(rec, dd)
            nss = []
            for h, n in ((0, n0), (1, n1)):
                ns = nsb.tile([HD, QB], bf16, tag=f"nst{h}", name=f"nst{h}")
                nc.vector.tensor_copy(ns, n[0:HD, :])
                nss.append(ns)
            prs = []
            for b in (0, 1):
                st_blk = pst.tile([128, 2 * QB], f32, tag="st", name="st_pr")
                prs.append((st_blk[:, 0:QB], st_blk[:, QB:2 * QB]))
            q0 = pacc.tile([128, D], f32, tag="qk", name="prq0")
            q1 = pacc.tile([128, D], f32, tag="qk", name="prq1")
            prs.append((q0, q1))

            def proj_mm(b):
                for h, ns, wo_s in ((0, nss[0], wo0_s), (1, nss[1], wo1_s)):
                    nc.tensor.matmul(prs[b][h],
                                     ns[:, b * 128:(b + 1) * 128],
                                     wo_s, start=True, stop=True)

            def scale_blk(b):
                t0 = small.tile([128, D], f32, tag="t0", name="t0")
                nc.scalar.mul(t0, prs[b][0], rec_t[:, b:b + 1])
                o_t = outp.tile([128, D], f32, tag="o", name="o_t")
                nc.vector.scalar_tensor_tensor(
                    o_t, prs[b][1], rec_t[:, 4 + b:4 + b + 1], t0,
                    mybir.AluOpType.mult, mybir.AluOpType.add)
                rows = slice(qb * QB + b * 128, qb * QB + (b + 1) * 128)
                nc.sync.dma_start(out=out[rows, :], in_=o_t)

            proj_mm(0)
            proj_mm(1)
            proj_mm(2)
            rec_tp = pacc.tile([128, 8], f32, tag="num", name="rec_tp")
            for c in range(8):
                nc.tensor.matmul(rec_tp[:, c:c + 1],
                                 rec[:, c * 128:(c + 1) * 128], one_f,
                                 start=True, stop=True)
            rec_t = small.tile([128, 8], f32, tag="rect", name="rec_t")
            nc.vector.tensor_copy(rec_t, rec_tp)
            scale_blk(0)
            st_blk = pst.tile([128, 2 * QB], f32, tag="st", name="st_pr")
            prs.append((st_blk[:, 0:QB], st_blk[:, QB:2 * QB]))
            proj_mm(3)
            scale_blk(1)
            scale_blk(2)
            scale_blk(3)

    nc.compile()
    return nc


def _get_compiled():
    global _compiled
    if _compiled is None:
        _compiled = _build()
    return _compiled


def kernel(gene_emb, expr_emb, V, M, fused_W, fused_b, Wq, bq, Wk, bk,
           out_W, out_b):
    gene_emb = np.asarray(gene_emb, dtype=np.float32)
    expr_emb = np.asarray(expr_emb, dtype=np.float32)
    V = np.asarray(V, dtype=np.float32)
    M = np.asarray(M, dtype=np.float32)
    fused_W = np.asarray(fused_W, dtype=np.float32)
    fused_b = np.asarray(fused_b, dtype=np.float32)
    Wq_ = np.asarray(Wq, dtype=np.float32)
    bq_ = np.asarray(bq, dtype=np.float32)
    Wk_ = np.asarray(Wk, dtype=np.float32)
    bk_ = np.asarray(bk, dtype=np.float32)
    out_W = np.asarray(out_W, dtype=np.float32)
    out_b = np.asarray(out_b, dtype=np.float32)

    nc = _get_compiled()

    import ml_dtypes
    m_np = np.float32 if M_DT == f32r else ml_dtypes.bfloat16
    p_np = np.float32 if P_DT == f32r else ml_dtypes.bfloat16
    x_np = np.float32 if X_DT == f32r else ml_dtypes.bfloat16

    fW = fused_W.astype(np.float64)
    Wqc = (fW @ Wq_.astype(np.float64)) * SCALE
    bqc = (fused_b.astype(np.float64) @ Wq_.astype(np.float64) + bq_) * SCALE
    Wkc = fW @ Wk_.astype(np.float64)
    bkc = fused_b.astype(np.float64) @ Wk_.astype(np.float64) + bk_

    def chunk_major(a, nch):  # [nch*128, F] -> [128, nch, F]
        F = a.shape[1]
        return np.ascontiguousarray(
            a.reshape(nch, 128, F).transpose(1, 0, 2))

    xt_b, mt_b = [], []
    for b in range(B):
        XT = np.concatenate([gene_emb[b], expr_emb[b]], axis=-1).T  # [1024,S]
        xg = XT.reshape(NCH, 128, NG, QB).transpose(1, 2, 0, 3)
        xt_b.append(np.ascontiguousarray(
            xg.reshape(128, NG * NCH, QB)).astype(x_np))
        # pre-tile the transposed mask: [T=(qb,kt), 128 keys, 512 queries]
        mtt = M[b].T.reshape(NKT, KT, NQB, QB).transpose(2, 0, 1, 3)
        mt_b.append(np.ascontiguousarray(
            mtt.reshape(NT, KT, QB)).astype(m_np))

    ones_col = np.ones((S, 1), np.float32)
    in_maps = []
    for d in range(N_CORES):
        b, p = d // 4, d % 4
        h0 = 2 * p
        cols = slice(p * 128, (p + 1) * 128)
        vs = []
        for h in (h0, h0 + 1):
            Vh = np.concatenate([V[b, :, h, :], ones_col], axis=1)  # [S,65]
            vs.append(chunk_major(Vh, NKT).astype(p_np))
        in_maps.append({
            "xt": xt_b[b],
            "mt": mt_b[b],
            "wq": chunk_major(Wqc[:, cols].astype(np.float32),
                              NCH).astype(x_np),
            "wk": chunk_major(Wkc[:, cols].astype(np.float32),
                              NCH).astype(x_np),
            "bq": bqc[cols].astype(np.float32).reshape(128, 1),
            "bk": bkc[cols].astype(np.float32).reshape(128, 1),
            "v0": vs[0],
            "v1": vs[1],
            "wo0": np.ascontiguousarray(
                out_W[h0 * HD:(h0 + 1) * HD, :]).astype(ml_dtypes.bfloat16),
            "wo1": np.ascontiguousarray(
                out_W[(h0 + 1) * HD:(h0 + 2) * HD, :]).astype(
                    ml_dtypes.bfloat16),
        })

    global _last_results
    n_run = int(os.environ.get("KERNEL_CORES", N_CORES))
    if n_run < N_CORES:
        in_maps = in_maps[:1] * N_CORES
    res = run_bass_kernel_spmd(nc, in_maps[:n_run],
                               core_ids=list(range(n_run)))
    if n_run < N_CORES:
        res.results = list(res.results) * (N_CORES // n_run)
    _last_results = res

    final = np.broadcast_to(out_b, (B, S, D)).astype(np.float32).copy()
    for d in range(N_CORES):
        final[d // 4] += res.results[d]["out"]
    return final
